# revision 4
# baseline (speedup 1.0000x reference)
"""Trainium2 Bass kernel for the constrained leaky RNN.

Model (reference):
    x_proj = inputs @ W_in.T + b_in                    # [B, T, H]
    h_t    = (1-a)*h_{t-1} + a*tanh(x_proj_t + h_{t-1} @ W_rec.T + h_bias)
    out    = hs @ W_out.T + b_out                      # [B, T, O]
with B=64, T=2048, I=128, H=512, O=64, a=0.2.

Strategy:
  - Data-parallel over batch: 8 cores x 8 batch rows, no collectives.
  - All on-chip state kept transposed: H on partitions (4 tiles of 128),
    batch (8) on the free dim -> per-step elementwise ops are [128, 4*8].
  - State rescale trick: store Hs = h / a. Then
        Hs_t = 0.8 * Hs_{t-1} + tanh(...)      (one fused scalar_tensor_tensor)
    with a folded into W_rec and W_out on the host.
  - Per step: 16 matmuls (K=128 tiles of 0.2*W_rec^T as stationary, 8-col
    rhs = previous state), one DVE add (psum + x_proj), one ACT tanh, one
    fused DVE blend.
  - x_proj precomputed per 256-step chunk (batched matmuls, bias folded in
    via the ACT copy), output projection batched per chunk.
"""

import os
import sys

sys.path.insert(0, "/opt/trn_rl_repo")

import numpy as np

B, T, I, H, O = 64, 2048, 128, 512, 64
NCORES = 8
BL = B // NCORES          # batch rows per core
ALPHA = 0.2
DECAY = 1.0 - ALPHA
TC = 256                  # steps per chunk
NCHUNK = T // TC
SUB = 64                  # steps per psum group in projections (64*8 = 512 cols)

DT_REC = "fp16"           # "fp32" | "bf16" | "fp16": W_rec / state / W_out path

_BUILD_CACHE = {}


def _build(dt_flag: str):
    import concourse.bass as bass
    import concourse.tile as tile
    from concourse import bacc, mybir
    from contextlib import ExitStack

    f32 = mybir.dt.float32
    dt_rec = {"fp32": f32, "bf16": mybir.dt.bfloat16,
              "fp16": mybir.dt.float16}[dt_flag]
    Alu = mybir.AluOpType
    Act = mybir.ActivationFunctionType

    nc = bacc.Bacc("TRN2")
    xT = nc.dram_tensor("xT", [I, T * BL], f32, kind="ExternalInput")
    wrecT = nc.dram_tensor("wrecT", [H, H], dt_rec, kind="ExternalInput")
    winT = nc.dram_tensor("winT", [I, H], f32, kind="ExternalInput")
    bias_c = nc.dram_tensor("bias_c", [128, 4], f32, kind="ExternalInput")
    woutT = nc.dram_tensor("woutT", [H, O], dt_rec, kind="ExternalInput")
    bout = nc.dram_tensor("bout", [O, 1], f32, kind="ExternalInput")
    outT = nc.dram_tensor("outT", [O, T * BL], f32, kind="ExternalOutput")

    with ExitStack() as ctx:
        tc = ctx.enter_context(tile.TileContext(nc))
        const = ctx.enter_context(tc.tile_pool(name="const", bufs=1))
        xpool = ctx.enter_context(tc.tile_pool(name="xpool", bufs=2))
        xppool = ctx.enter_context(tc.tile_pool(name="xppool", bufs=2))
        hpool = ctx.enter_context(tc.tile_pool(name="hpool", bufs=2))
        tmp = ctx.enter_context(tc.tile_pool(name="tmp", bufs=4))
        opool = ctx.enter_context(tc.tile_pool(name="opool", bufs=2))
        ps_h = ctx.enter_context(tc.tile_pool(name="ps_h", bufs=2, space="PSUM"))
        ps_xp = ctx.enter_context(tc.tile_pool(name="ps_xp", bufs=2, space="PSUM"))
        ps_o = ctx.enter_context(tc.tile_pool(name="ps_o", bufs=2, space="PSUM"))

        # ---- constants ----
        wrec_sb = const.tile([128, 4, H], dt_rec)       # [:, i, j*128+m]
        for i in range(4):
            nc.sync.dma_start(wrec_sb[:, i], wrecT[i * 128:(i + 1) * 128, :])
        win_sb = const.tile([I, H], f32)
        nc.sync.dma_start(win_sb, winT[:, :])
        bias_sb = const.tile([128, 4], f32)
        nc.sync.dma_start(bias_sb, bias_c[:, :])
        wout_sb = const.tile([128, 4, O], dt_rec)
        for j in range(4):
            nc.sync.dma_start(wout_sb[:, j], woutT[j * 128:(j + 1) * 128, :])
        bout_sb = const.tile([O, 1], f32)
        nc.sync.dma_start(bout_sb, bout[:, :])

        h_init = const.tile([128, 4, BL], dt_rec)
        nc.any.memzero(h_init[:])

        prev = h_init[:]                                 # state at t-1: [128, 4, BL]
        for c in range(NCHUNK):
            x_sb = xpool.tile([I, TC * BL], f32, tag="x")
            nc.sync.dma_start(x_sb, xT[:, c * TC * BL:(c + 1) * TC * BL])

            # x_proj for the chunk: xp[p, t, j, b] (+ combined bias)
            xp_sb = xppool.tile([128, TC, 4, BL], f32, tag="xp")
            for sub in range(TC // SUB):
                for j in range(4):
                    psx = ps_xp.tile([128, SUB * BL], f32, tag="psxp")
                    nc.tensor.matmul(
                        psx,
                        win_sb[:, j * 128:(j + 1) * 128],
                        x_sb[:, sub * SUB * BL:(sub + 1) * SUB * BL],
                        start=True, stop=True,
                    )
                    nc.scalar.activation(
                        xp_sb[:, sub * SUB:(sub + 1) * SUB, j, :],
                        psx.rearrange("p (t b) -> p t b", b=BL),
                        Act.Identity,
                        bias=bias_sb[:, j:j + 1],
                    )

            # recurrence
            hs = hpool.tile([128, TC, 4, BL], dt_rec, tag="hs")
            for t in range(TC):
                ps = ps_h.tile([128, 4 * BL], f32, tag="psh")
                psv = ps.rearrange("p (j b) -> p j b", b=BL)
                for j in range(4):
                    for i in range(4):
                        nc.tensor.matmul(
                            psv[:, j],
                            wrec_sb[:, i, j * 128:(j + 1) * 128],
                            prev[:, i],
                            start=(i == 0), stop=(i == 3),
                        )
                pre = tmp.tile([128, 4, BL], f32, tag="pre")
                nc.vector.tensor_add(out=pre[:], in0=psv, in1=xp_sb[:, t])
                g = tmp.tile([128, 4, BL], dt_rec, tag="g")
                nc.scalar.activation(g[:], pre[:], Act.Tanh)
                nc.vector.scalar_tensor_tensor(
                    out=hs[:, t], in0=prev, scalar=DECAY, in1=g[:],
                    op0=Alu.mult, op1=Alu.add,
                )
                prev = hs[:, t]

            # output projection for the chunk
            for sub in range(TC // SUB):
                pso = ps_o.tile([O, SUB * BL], f32, tag="pso")
                for j in range(4):
                    nc.tensor.matmul(
                        pso,
                        wout_sb[:, j],
                        hs[:, sub * SUB:(sub + 1) * SUB, j, :],
                        start=(j == 0), stop=(j == 3),
                    )
                ob = opool.tile([O, SUB * BL], f32, tag="ob")
                nc.scalar.activation(ob[:], pso[:], Act.Identity,
                                     bias=bout_sb[:, 0:1])
                nc.sync.dma_start(
                    outT[:, (c * TC + sub * SUB) * BL:(c * TC + (sub + 1) * SUB) * BL],
                    ob[:],
                )

    nc.finalize()
    return nc


def _get_nc(dt_flag: str):
    if dt_flag not in _BUILD_CACHE:
        _BUILD_CACHE[dt_flag] = _build(dt_flag)
    return _BUILD_CACHE[dt_flag]


def _prep_in_maps(inputs, dt_flag: str):
    import ml_dtypes

    x = np.asarray(inputs["inputs"], dtype=np.float32)
    W_in = np.asarray(inputs["W_in"], dtype=np.float32)
    b_in = np.asarray(inputs["b_in"], dtype=np.float32)
    W_rec = np.asarray(inputs["W_rec"], dtype=np.float32)
    h_bias = np.asarray(inputs["h_bias"], dtype=np.float32)
    W_out = np.asarray(inputs["W_out"], dtype=np.float32)
    b_out = np.asarray(inputs["b_out"], dtype=np.float32)

    dt = {"fp32": np.float32, "bf16": ml_dtypes.bfloat16,
          "fp16": np.float16}[dt_flag]
    wrecT = np.ascontiguousarray((ALPHA * W_rec.T).astype(dt))
    winT = np.ascontiguousarray(W_in.T)
    bias_c = np.ascontiguousarray((b_in + h_bias).reshape(4, 128).T)
    woutT = np.ascontiguousarray((ALPHA * W_out.T).astype(dt))
    bout = np.ascontiguousarray(b_out.reshape(O, 1))

    in_maps = []
    for c in range(NCORES):
        xc = x[c * BL:(c + 1) * BL]                     # [BL, T, I]
        xTc = np.ascontiguousarray(xc.transpose(2, 1, 0).reshape(I, T * BL))
        in_maps.append({
            "xT": xTc, "wrecT": wrecT, "winT": winT,
            "bias_c": bias_c, "woutT": woutT, "bout": bout,
        })
    return in_maps


def _run(inputs, trace=False, dt_flag=None, tmpdir=None):
    from concourse import bass_utils

    if dt_flag is None:
        dt_flag = DT_REC
    nc = _get_nc(dt_flag)
    in_maps = _prep_in_maps(inputs, dt_flag)
    res = bass_utils.run_bass_kernel_spmd(
        nc, in_maps, core_ids=list(range(NCORES)), trace=trace, tmpdir=tmpdir,
    )
    outs = []
    for c in range(NCORES):
        oT = res.results[c]["outT"]                     # [O, T*BL]
        outs.append(oT.reshape(O, T, BL).transpose(2, 1, 0))
    full = np.concatenate(outs, axis=0).astype(np.float32)
    return full, res


def kernel(**inputs) -> np.ndarray:
    out, _ = _run(inputs, trace=False)
    return out


# revision 6
# speedup vs baseline: 20.8500x; 20.8500x over previous
"""Trainium2 Bass kernel for the constrained leaky RNN.

Model (reference):
    x_proj = inputs @ W_in.T + b_in                    # [B, T, H]
    h_t    = (1-a)*h_{t-1} + a*tanh(x_proj_t + h_{t-1} @ W_rec.T + h_bias)
    out    = hs @ W_out.T + b_out                      # [B, T, O]
with B=64, T=2048, I=128, H=512, O=64, a=0.2.

Strategy:
  - Data-parallel over batch: 8 cores x 8 batch rows, no collectives.
  - All on-chip state kept transposed: H on partitions (4 tiles of 128),
    batch (8) on the free dim -> per-step elementwise ops are [128, 4*8].
  - State rescale trick: store Hs = h / a. Then
        Hs_t = 0.8 * Hs_{t-1} + tanh(...)      (one fused scalar_tensor_tensor)
    with a folded into W_rec and W_out on the host.
  - Per step: 16 matmuls (K=128 tiles of 0.2*W_rec^T as stationary, 8-col
    rhs = previous state), one DVE add (psum + x_proj), one ACT tanh, one
    fused DVE blend.
  - x_proj precomputed per 256-step chunk (batched matmuls, bias folded in
    via the ACT copy), output projection batched per chunk.
"""

import os
import sys

sys.path.insert(0, "/opt/trn_rl_repo")

import numpy as np

B, T, I, H, O = 64, 2048, 128, 512, 64
NCORES = 8
BL = B // NCORES          # batch rows per core
ALPHA = 0.2
DECAY = 1.0 - ALPHA
TC = 256                  # steps per chunk
NCHUNK = T // TC
SUB = 64                  # steps per psum group in projections (64*8 = 512 cols)

DT_REC = "fp16"           # "fp32" | "bf16" | "fp16": W_rec / state / W_out path
REPEAT = 1                # timing amplification: run the whole computation N times

_BUILD_CACHE = {}


def _build(dt_flag: str):
    import concourse.bass as bass
    import concourse.tile as tile
    from concourse import bacc, mybir
    from contextlib import ExitStack

    f32 = mybir.dt.float32
    dt_rec = {"fp32": f32, "bf16": mybir.dt.bfloat16,
              "fp16": mybir.dt.float16}[dt_flag]
    Alu = mybir.AluOpType
    Act = mybir.ActivationFunctionType

    nc = bacc.Bacc("TRN2")
    xT = nc.dram_tensor("xT", [I, T * BL], f32, kind="ExternalInput")
    wrecT = nc.dram_tensor("wrecT", [H, H], dt_rec, kind="ExternalInput")
    winT = nc.dram_tensor("winT", [I, H], f32, kind="ExternalInput")
    bias_c = nc.dram_tensor("bias_c", [128, 4], f32, kind="ExternalInput")
    woutT = nc.dram_tensor("woutT", [H, O], dt_rec, kind="ExternalInput")
    bout = nc.dram_tensor("bout", [O, 1], f32, kind="ExternalInput")
    outT = nc.dram_tensor("outT", [O, T * BL], f32, kind="ExternalOutput")

    with ExitStack() as ctx:
        tc = ctx.enter_context(tile.TileContext(nc))
        const = ctx.enter_context(tc.tile_pool(name="const", bufs=1))
        xpool = ctx.enter_context(tc.tile_pool(name="xpool", bufs=2))
        xppool = ctx.enter_context(tc.tile_pool(name="xppool", bufs=2))
        hpool = ctx.enter_context(tc.tile_pool(name="hpool", bufs=2))
        tmp = ctx.enter_context(tc.tile_pool(name="tmp", bufs=4))
        opool = ctx.enter_context(tc.tile_pool(name="opool", bufs=2))
        ps_h = ctx.enter_context(tc.tile_pool(name="ps_h", bufs=2, space="PSUM"))
        ps_xp = ctx.enter_context(tc.tile_pool(name="ps_xp", bufs=2, space="PSUM"))
        ps_o = ctx.enter_context(tc.tile_pool(name="ps_o", bufs=2, space="PSUM"))

        # ---- constants ----
        wrec_sb = const.tile([128, 4, H], dt_rec)       # [:, i, j*128+m]
        for i in range(4):
            nc.sync.dma_start(wrec_sb[:, i], wrecT[i * 128:(i + 1) * 128, :])
        win_sb = const.tile([I, H], f32)
        nc.sync.dma_start(win_sb, winT[:, :])
        bias_sb = const.tile([128, 4], f32)
        nc.sync.dma_start(bias_sb, bias_c[:, :])
        wout_sb = const.tile([128, 4, O], dt_rec)
        for j in range(4):
            nc.sync.dma_start(wout_sb[:, j], woutT[j * 128:(j + 1) * 128, :])
        bout_sb = const.tile([O, 1], f32)
        nc.sync.dma_start(bout_sb, bout[:, :])

        h_init = const.tile([128, 4, BL], dt_rec)
        nc.any.memzero(h_init[:])

        for _rep in range(REPEAT):
          prev = h_init[:]                               # state at t-1: [128, 4, BL]
          for c in range(NCHUNK):
            x_sb = xpool.tile([I, TC * BL], f32, tag="x")
            nc.sync.dma_start(x_sb, xT[:, c * TC * BL:(c + 1) * TC * BL])

            # x_proj for the chunk: xp[p, t, j, b] (+ combined bias)
            xp_sb = xppool.tile([128, TC, 4, BL], f32, tag="xp")
            for sub in range(TC // SUB):
                for j in range(4):
                    psx = ps_xp.tile([128, SUB * BL], f32, tag="psxp")
                    nc.tensor.matmul(
                        psx,
                        win_sb[:, j * 128:(j + 1) * 128],
                        x_sb[:, sub * SUB * BL:(sub + 1) * SUB * BL],
                        start=True, stop=True,
                    )
                    nc.scalar.activation(
                        xp_sb[:, sub * SUB:(sub + 1) * SUB, j, :],
                        psx.rearrange("p (t b) -> p t b", b=BL),
                        Act.Identity,
                        bias=bias_sb[:, j:j + 1],
                    )

            # recurrence
            hs = hpool.tile([128, TC, 4, BL], dt_rec, tag="hs")
            for t in range(TC):
                ps = ps_h.tile([128, 4 * BL], f32, tag="psh")
                psv = ps.rearrange("p (j b) -> p j b", b=BL)
                for j in range(4):
                    for i in range(4):
                        nc.tensor.matmul(
                            psv[:, j],
                            wrec_sb[:, i, j * 128:(j + 1) * 128],
                            prev[:, i],
                            start=(i == 0), stop=(i == 3),
                        )
                pre = tmp.tile([128, 4, BL], f32, tag="pre")
                nc.vector.tensor_add(out=pre[:], in0=psv, in1=xp_sb[:, t])
                g = tmp.tile([128, 4, BL], dt_rec, tag="g")
                nc.scalar.activation(g[:], pre[:], Act.Tanh)
                nc.vector.scalar_tensor_tensor(
                    out=hs[:, t], in0=prev, scalar=DECAY, in1=g[:],
                    op0=Alu.mult, op1=Alu.add,
                )
                prev = hs[:, t]

            # output projection for the chunk
            for sub in range(TC // SUB):
                pso = ps_o.tile([O, SUB * BL], f32, tag="pso")
                for j in range(4):
                    nc.tensor.matmul(
                        pso,
                        wout_sb[:, j],
                        hs[:, sub * SUB:(sub + 1) * SUB, j, :],
                        start=(j == 0), stop=(j == 3),
                    )
                ob = opool.tile([O, SUB * BL], f32, tag="ob")
                nc.scalar.activation(ob[:], pso[:], Act.Identity,
                                     bias=bout_sb[:, 0:1])
                nc.sync.dma_start(
                    outT[:, (c * TC + sub * SUB) * BL:(c * TC + (sub + 1) * SUB) * BL],
                    ob[:],
                )

    nc.finalize()
    return nc


def _get_nc(dt_flag: str):
    if dt_flag not in _BUILD_CACHE:
        _BUILD_CACHE[dt_flag] = _build(dt_flag)
    return _BUILD_CACHE[dt_flag]


def _prep_in_maps(inputs, dt_flag: str):
    import ml_dtypes

    x = np.asarray(inputs["inputs"], dtype=np.float32)
    W_in = np.asarray(inputs["W_in"], dtype=np.float32)
    b_in = np.asarray(inputs["b_in"], dtype=np.float32)
    W_rec = np.asarray(inputs["W_rec"], dtype=np.float32)
    h_bias = np.asarray(inputs["h_bias"], dtype=np.float32)
    W_out = np.asarray(inputs["W_out"], dtype=np.float32)
    b_out = np.asarray(inputs["b_out"], dtype=np.float32)

    dt = {"fp32": np.float32, "bf16": ml_dtypes.bfloat16,
          "fp16": np.float16}[dt_flag]
    wrecT = np.ascontiguousarray((ALPHA * W_rec.T).astype(dt))
    winT = np.ascontiguousarray(W_in.T)
    bias_c = np.ascontiguousarray((b_in + h_bias).reshape(4, 128).T)
    woutT = np.ascontiguousarray((ALPHA * W_out.T).astype(dt))
    bout = np.ascontiguousarray(b_out.reshape(O, 1))

    in_maps = []
    for c in range(NCORES):
        xc = x[c * BL:(c + 1) * BL]                     # [BL, T, I]
        xTc = np.ascontiguousarray(xc.transpose(2, 1, 0).reshape(I, T * BL))
        in_maps.append({
            "xT": xTc, "wrecT": wrecT, "winT": winT,
            "bias_c": bias_c, "woutT": woutT, "bout": bout,
        })
    return in_maps


def _run(inputs, trace=False, dt_flag=None, tmpdir=None):
    from concourse import bass_utils

    if dt_flag is None:
        dt_flag = DT_REC
    nc = _get_nc(dt_flag)
    in_maps = _prep_in_maps(inputs, dt_flag)
    res = bass_utils.run_bass_kernel_spmd(
        nc, in_maps, core_ids=list(range(NCORES)), trace=trace, tmpdir=tmpdir,
    )
    outs = []
    for c in range(NCORES):
        oT = res.results[c]["outT"]                     # [O, T*BL]
        outs.append(oT.reshape(O, T, BL).transpose(2, 1, 0))
    full = np.concatenate(outs, axis=0).astype(np.float32)
    return full, res


def kernel(**inputs) -> np.ndarray:
    out, _ = _run(inputs, trace=False)
    return out
